# revision 2
# baseline (speedup 1.0000x reference)
"""Trainium2 Bass kernel v2 for BitNet multi-group-query attention.

Problem: nn_BitnetMultiGroupQueryAttention_41755672052100
  B=4, S=2048, E=2048, QH=16, KH=4, HD=128, KVE=512, fp32.

Differences vs v1 baseline:
  * Weight quantization (static model prep) happens on host: ternary
    int-grid weights ship as bf16, Wq group-summed to [E,512]; scales ship
    as a tiny params vector. Removes ~38MB/core DMA and the serial
    weight-quant header.
  * Activations ship as bf16 (wire format) - halves act-load DMA.
  * Attention is all-bf16 on the PE (fp32 matmuls cost 4x): probs bf16,
    vS bf16 (cv folded), z row-sums via bf16 ones-matmuls accumulating in
    a shared [4,512] PSUM bank (exact fp32 accumulation).
  * Causal mask multiplies only on tiles that can be partial/masked
    (j in 0..7 for the low block, 8..15 for the high block), on DVE.
  * Heads processed 4-together per key tile so PV matmuls never wait on
    the exp chain.
  * LN/out-proj tail transposes via the PE (no DRAM bounce), out written
    per 128-token row block.

Sharding: core c -> batch b=c//2, two 512-token query blocks ({0,3} even
half, {1,2} odd half; balanced causal work). SPMD identical program;
per-core behavior differs only through data (thr causal thresholds).
"""

import os
import sys

for _p in ("/opt/trn_rl_repo", "/root/.axon_site/_ro/trn_rl_repo"):
    if os.path.isdir(_p) and _p not in sys.path:
        sys.path.insert(0, _p)
        break

import numpy as np
import ml_dtypes

BF16 = ml_dtypes.bfloat16

B, S, E = 4, 2048, 2048
QH, KH = 16, 4
HD, KVE = 128, 512
NCORES = 8
BLKS = [[0, 3], [1, 2]]        # global 512-token block ids per half
NT_Q = 1024                    # query tokens per core
MAGIC = 12582912.0             # 1.5 * 2**23 : fp32 RNE rounding constant
LN_EPS = 1e-5
CHUNK = 512

_CACHE = {}


def _build(flags, debug=False):
    has_bv, has_bo, has_gamma, has_beta = flags

    import concourse.bass as bass
    import concourse.tile as tile
    import concourse.mybir as mybir
    from concourse import bacc
    from concourse.masks import make_identity

    f32 = mybir.dt.float32
    bf16 = mybir.dt.bfloat16
    i32 = mybir.dt.int32
    ALU = mybir.AluOpType
    ACTF = mybir.ActivationFunctionType
    AX = mybir.AxisListType

    nc = bacc.Bacc(None, target_bir_lowering=False)

    # ---------------- DRAM I/O ----------------
    f16 = mybir.dt.float16
    q_in = nc.dram_tensor("q_in", [NT_Q, E], f16, kind="ExternalInput").ap()
    k_in = nc.dram_tensor("k_in", [S, E], f16, kind="ExternalInput").ap()
    v_in = nc.dram_tensor("v_in", [S, E], f16, kind="ExternalInput").ap()
    wqs_d = nc.dram_tensor("wqs", [E, KVE], bf16, kind="ExternalInput").ap()
    wkq_d = nc.dram_tensor("wkq", [E, KVE], bf16, kind="ExternalInput").ap()
    wvq_d = nc.dram_tensor("wvq", [E, KVE], bf16, kind="ExternalInput").ap()
    woq_d = nc.dram_tensor("woq", [KVE, E], bf16, kind="ExternalInput").ap()
    scl_d = nc.dram_tensor("scl", [8], f32, kind="ExternalInput").ap()
    bqs_d = nc.dram_tensor("bqs", [128, KH], f32, kind="ExternalInput").ap()
    thr_d = nc.dram_tensor("thr", [2, 512], f32, kind="ExternalInput").ap()
    bv_d = nc.dram_tensor("bv", [KVE], f32, kind="ExternalInput").ap()
    bo_d = nc.dram_tensor("bo", [E], f32, kind="ExternalInput").ap()
    gamma_d = nc.dram_tensor("gamma", [KVE], f32, kind="ExternalInput").ap()
    beta_d = nc.dram_tensor("beta", [KVE], f32, kind="ExternalInput").ap()
    out_d = nc.dram_tensor("out", [NT_Q, E], f32, kind="ExternalOutput").ap()
    if debug:
        dbg_kT = nc.dram_tensor("dbg_kT", [128, S], f32,
                                kind="ExternalOutput").ap()
        dbg_qT = nc.dram_tensor("dbg_qT", [128, NT_Q], f32,
                                kind="ExternalOutput").ap()
        dbg_vS = nc.dram_tensor("dbg_vS", [128, KVE], f32,
                                kind="ExternalOutput").ap()
        dbg_xT = nc.dram_tensor("dbg_xT", [128, NT_Q], f32,
                                kind="ExternalOutput").ap()

    def bcast_ap(src_ap, parts=128):
        # DMA-replicate a free-only DRAM AP across `parts` partitions
        return bass.AP(
            tensor=src_ap.tensor,
            offset=src_ap.offset,
            ap=[[0, parts]] + list(src_ap.ap),
        )

    with tile.TileContext(nc) as tc:
      with tc.tile_pool(name="persist", bufs=1) as PP, \
           tc.tile_pool(name="dram", bufs=1, space="DRAM") as DR:
        # ---------- constants ----------
        ones_bf = PP.tile([128, 1], bf16, tag="ones_bf")
        nc.vector.memset(ones_bf, 1.0)
        ones_row = PP.tile([1, 128], f32, tag="ones_row")
        nc.vector.memset(ones_row, 1.0)
        eps_col = PP.tile([128, 1], f32, tag="eps_col")
        nc.vector.memset(eps_col, LN_EPS)
        magic_col = PP.tile([128, 1], f32, tag="magic_col")
        nc.vector.memset(magic_col, MAGIC)
        negmagic_col = PP.tile([128, 1], f32, tag="negmagic_col")
        nc.vector.memset(negmagic_col, -MAGIC)
        ident = PP.tile([128, 128], f32, tag="ident")
        make_identity(nc, ident)
        ident_bf = PP.tile([128, 128], bf16, tag="ident_bf")
        nc.gpsimd.tensor_copy(ident_bf, ident)
        sj_i = PP.tile([128, 16], i32, tag="sj_i")
        # sj[p, j] = p + 128*j  (global key index of partition p in s-tile j)
        nc.gpsimd.iota(sj_i, pattern=[[128, 16]], base=0, channel_multiplier=1)
        sj = PP.tile([128, 16], f32, tag="sj")
        nc.vector.tensor_copy(sj, sj_i)

        clip_k = PP.tile([128, 16], f32, tag="clip_k")
        clip_v = PP.tile([128, 16], f32, tag="clip_v")
        ck_all = PP.tile([128, 16], f32, tag="ck_all")
        cv_all = PP.tile([128, 16], f32, tag="cv_all")
        co_all = PP.tile([128, 8], f32, tag="co_all")

        scl_bc = PP.tile([128, 8], f32, tag="scl_bc")
        nc.gpsimd.dma_start(out=scl_bc, in_=bcast_ap(scl_d))
        bqs = PP.tile([128, KH], f32, tag="bqs")
        nc.gpsimd.dma_start(out=bqs, in_=bqs_d)
        thr_bc = [PP.tile([128, 512], f32, tag=f"thr{lb}", name=f"thr{lb}")
                  for lb in range(2)]
        for lb in range(2):
            nc.gpsimd.dma_start(out=thr_bc[lb], in_=bcast_ap(thr_d[lb]))

        # ---------- weights (host-quantized int-grid bf16) ----------
        wqs = PP.tile([128, 16, KVE], bf16, tag="wqs")
        wkq = PP.tile([128, 16, KVE], bf16, tag="wkq")
        wvq = PP.tile([128, 16, KVE], bf16, tag="wvq")
        woq = PP.tile([128, 4, E], bf16, tag="woq")
        nc.gpsimd.dma_start(
            out=wqs, in_=wqs_d.rearrange("(j p) f -> p j f", p=128))

        if has_bv:
            bv_bc = PP.tile([128, KVE], f32, tag="bv_bc")
            nc.gpsimd.dma_start(out=bv_bc, in_=bcast_ap(bv_d))
        if has_bo:
            bo_bc = PP.tile([128, E], f32, tag="bo_bc")
            nc.gpsimd.dma_start(out=bo_bc, in_=bcast_ap(bo_d))
        if has_gamma:
            gamma_bc = PP.tile([128, KVE], f32, tag="gamma_bc")
            nc.gpsimd.dma_start(out=gamma_bc, in_=bcast_ap(gamma_d))
        if has_beta:
            beta_bc = PP.tile([128, KVE], f32, tag="beta_bc")
            nc.gpsimd.dma_start(out=beta_bc, in_=bcast_ap(beta_d))

        # persistent activations
        kT = [PP.tile([128, S], bf16, tag=f"kT{h}", name=f"kT{h}")
              for h in range(KH)]                  # [d, s] int sums
        vS = [PP.tile([128, KVE], bf16, tag=f"v{j}", name=f"v{j}")
              for j in range(16)]                  # [s, dv] cv-folded
        qT = [PP.tile([128, NT_Q], bf16, tag=f"qT{h}", name=f"qT{h}")
              for h in range(KH)]                  # [d, n] cq-folded
        xT = [PP.tile([128, NT_Q], f32, tag=f"xT{h}", name=f"xT{h}")
              for h in range(KH)]                  # [dv, n] attention out

        # ---------------- stage 1: act quant + transpose + proj ----------
        with tc.tile_pool(name="aload", bufs=3) as AL, \
             tc.tile_pool(name="aq_t1", bufs=3) as AQ, \
             tc.tile_pool(name="aq_small", bufs=8) as AS, \
             tc.tile_pool(name="achunk", bufs=2) as AC, \
             tc.tile_pool(name="proj_psum", bufs=3, space="PSUM") as PJ, \
             tc.tile_pool(name="proj_psum2", bufs=3, space="PSUM") as PJ2:

            def quant_chunk(src_dram, c0, kind):
                """Load+quantize CHUNK fp16 tokens at row c0; per-tile
                SBUF->SBUF xbar transposes build chunk [128,16,CHUNK]."""
                chunk = AC.tile([128, 16, CHUNK], bf16, tag="chunk")
                for ti in range(CHUNK // 128):
                    tok0 = c0 + ti * 128
                    jt = tok0 // 128
                    xt = AL.tile([128, E], f16, tag="aload")
                    # loads live alone on the sync ring: they have no deps,
                    # so the ring never head-of-line blocks and prefetch
                    # runs arbitrarily far ahead
                    nc.sync.dma_start(
                        out=xt, in_=src_dram[tok0:tok0 + 128, :])
                    mx = AS.tile([128, 1], f32, tag="aq_mx")
                    nc.vector.tensor_reduce(
                        mx, xt, axis=AX.X, op=ALU.max,
                        apply_absolute_value=True)
                    # clip straight into the persistent per-token-tile col
                    if kind == "k":
                        clip = clip_k[:, jt:jt + 1]
                    elif kind == "v":
                        clip = clip_v[:, jt:jt + 1]
                    else:
                        clip = AS.tile([128, 1], f32, tag="aq_clip")
                    nc.vector.tensor_scalar(clip, mx, 1e-5, None, op0=ALU.max)
                    sx = AS.tile([128, 1], f32, tag="aq_sx")
                    nc.vector.reciprocal(sx, clip)
                    nc.vector.tensor_scalar(sx, sx, 127.0, None, op0=ALU.mult)
                    t1 = AQ.tile([128, E], f32, tag="aq_t1")
                    nc.scalar.activation(
                        out=t1, in_=xt, func=ACTF.Identity,
                        bias=magic_col, scale=sx)
                    t2 = AQ.tile([128, E], bf16, tag="aq_t2")
                    if kind == "q":
                        cq = AS.tile([128, 1], f32, tag="aq_cq")
                        nc.vector.tensor_scalar(
                            cq, clip, scl_bc[:, 0:1], None, op0=ALU.mult)
                        nc.gpsimd.tensor_scalar(
                            t2, t1, -MAGIC, cq, op0=ALU.add, op1=ALU.mult)
                    else:
                        nc.gpsimd.tensor_scalar(
                            t2, t1, -MAGIC, None, op0=ALU.add)
                    nc.scalar.dma_start_transpose(
                        out=chunk[:, :, ti * 128:(ti + 1) * 128], in_=t2)
                return chunk

            def proj_q(chunk, qi):
                c0 = qi * CHUNK
                for h in range(KH):
                    ps = PJ.tile([128, CHUNK], f32, tag="proj_ps")
                    for e in range(16):
                        nc.tensor.matmul(
                            ps,
                            lhsT=wqs[:, e, h * 128:(h + 1) * 128],
                            rhs=chunk[:, e, :],
                            start=(e == 0), stop=(e == 15),
                        )
                    nc.scalar.activation(
                        out=qT[h][:, c0:c0 + CHUNK], in_=ps,
                        func=ACTF.Identity, bias=bqs[:, h:h + 1], scale=1.0)

            def proj_k(chunk, ci):
                c0 = ci * CHUNK
                for h in range(KH):
                    ps = PJ.tile([128, CHUNK], f32, tag="proj_ps")
                    for e in range(16):
                        nc.tensor.matmul(
                            ps,
                            lhsT=wkq[:, e, h * 128:(h + 1) * 128],
                            rhs=chunk[:, e, :],
                            start=(e == 0), stop=(e == 15),
                        )
                    nc.vector.tensor_copy(kT[h][:, c0:c0 + CHUNK], ps)

            def proj_v(chunk, ci):
                c0 = ci * CHUNK
                j0 = c0 // 128
                nc.vector.tensor_scalar(
                    cv_all[:, j0:j0 + 4], clip_v[:, j0:j0 + 4],
                    scl_bc[:, 2:3], None, op0=ALU.mult)
                for ti in range(CHUNK // 128):
                    jt = j0 + ti
                    ps = PJ2.tile([128, KVE], f32, tag="proj_ps_v")
                    for e in range(16):
                        nc.tensor.matmul(
                            ps,
                            lhsT=chunk[:, e, ti * 128:(ti + 1) * 128],
                            rhs=wvq[:, e, :],
                            start=(e == 0), stop=(e == 15),
                        )
                    nc.vector.tensor_scalar(
                        vS[jt], ps, cv_all[:, jt:jt + 1], None, op0=ALU.mult)
                    if has_bv:
                        nc.vector.tensor_add(vS[jt], vS[jt], bv_bc)

            # Software-pipelined emission: chunk N+1's quant ops are
            # emitted BEFORE chunk N's projections, so the engine queues
            # (DVE/ACT/Pool) never park quant work behind PSUM epilogues
            # that wait on matmuls. Weight loads (SWDGE) are staggered so
            # the first chunks' act loads own the sync ring.
            plan = [("q", 0), ("q", 1), ("k", 0), ("v", 0), ("k", 1),
                    ("v", 1), ("k", 2), ("v", 2), ("k", 3), ("v", 3)]
            srcs = {"q": q_in, "k": k_in, "v": v_in}
            projs = {"q": proj_q, "k": proj_k, "v": proj_v}
            wload = {1: (wkq, wkq_d, "(j p) f -> p j f"),
                     2: (wvq, wvq_d, "(j p) f -> p j f"),
                     3: (woq, woq_d, "(c p) f -> p c f")}
            prev = None
            for i, (kind, ci) in enumerate(plan):
                ch = quant_chunk(srcs[kind], ci * CHUNK, kind)
                if prev is not None:
                    pk, pc, pch = prev
                    projs[pk](pch, pc)
                if i in wload:
                    wt, wd, pat = wload[i]
                    nc.gpsimd.dma_start(out=wt, in_=wd.rearrange(pat, p=128))
                prev = (kind, ci, ch)
            projs[prev[0]](prev[2], prev[1])

        # ck columns for the exp stage
        nc.vector.tensor_scalar(
            ck_all, clip_k, scl_bc[:, 1:2], None, op0=ALU.mult)

        # ---------------- stage 2: attention ----------------
        # local block 0 attends keys < 1024 (8 j-tiles), block 1 all 2048.
        # Mask multiplies only where a tile can be partial or fully masked
        # on either core of the pair: j in 0..7 for lb=0, 8..15 for lb=1.
        NJ = [8, 16]
        MASKED = [range(0, 8), range(8, 16)]
        with tc.tile_pool(name="amask", bufs=1) as MP, \
             tc.tile_pool(name="aprobs", bufs=6) as PB, \
             tc.tile_pool(name="azrow", bufs=2) as ZR, \
             tc.tile_pool(name="sim_psum", bufs=3, space="PSUM") as SP_, \
             tc.tile_pool(name="x_psum", bufs=1, space="PSUM") as XP, \
             tc.tile_pool(name="z_psum", bufs=1, space="PSUM") as ZP:
            masks = {}
            for lb in range(2):
                for j in MASKED[lb]:
                    m = MP.tile([128, 512], bf16, tag=f"mask{j}",
                                name=f"mask{j}_{lb}")
                    # mask[p, n] = (thr[lb, n] >= p + 128*j)
                    nc.vector.tensor_scalar(
                        m, thr_bc[lb], sj[:, j:j + 1], None, op0=ALU.is_ge)
                    masks[(lb, j)] = m

            for lb in range(2):
                nj = NJ[lb]
                ps_x = [XP.tile([128, 512], f32, tag=f"ps_x{h}",
                                name=f"ps_x{h}_{lb}") for h in range(KH)]
                ps_z = ZP.tile([128, 512], f32, tag="ps_z", name=f"ps_z_{lb}")
                for j in range(nj):
                    sims = []
                    for h in range(KH):
                        ps_s = SP_.tile([128, 512], f32, tag="ps_s")
                        nc.tensor.matmul(
                            ps_s,
                            lhsT=kT[h][:, j * 128:(j + 1) * 128],
                            rhs=qT[h][:, lb * 512:(lb + 1) * 512],
                            start=True, stop=True,
                        )
                        sims.append(ps_s)
                    probs = []
                    for h in range(KH):
                        p = PB.tile([128, 512], bf16, tag="probs")
                        nc.scalar.activation(
                            out=p, in_=sims[h], func=ACTF.Exp,
                            scale=ck_all[:, j:j + 1])
                        if (lb, j) in masks:
                            nc.vector.tensor_mul(p, p, masks[(lb, j)])
                        probs.append(p)
                    for h in range(KH):
                        nc.tensor.matmul(
                            ps_x[h],
                            lhsT=vS[j][:, h * 128:(h + 1) * 128],
                            rhs=probs[h],
                            start=(j == 0), stop=(j == nj - 1),
                        )
                        nc.tensor.matmul(
                            ps_z[h * 32:h * 32 + 1, :],
                            lhsT=ones_bf,
                            rhs=probs[h],
                            start=(j == 0), stop=(j == nj - 1),
                            tile_position=(0, h * 32),
                        )
                invz = ZR.tile([1, 4 * 512], f32, tag="invz")
                for h in range(KH):
                    nc.vector.reciprocal(
                        invz[0:1, h * 512:(h + 1) * 512],
                        ps_z[h * 32:h * 32 + 1, :])
                for h in range(KH):
                    ps_b = SP_.tile([128, 512], f32, tag="ps_s",
                                    name=f"ps_b{h}_{lb}")
                    nc.tensor.matmul(ps_b, lhsT=ones_row,
                                     rhs=invz[0:1, h * 512:(h + 1) * 512],
                                     start=True, stop=True)
                    invz_bc = ZR.tile([128, 512], f32, tag="invz_bc")
                    nc.vector.tensor_copy(invz_bc, ps_b)
                    nc.vector.tensor_mul(
                        xT[h][:, lb * 512:(lb + 1) * 512], ps_x[h], invz_bc)

        # ---------------- stage 3: LN + out quant + out proj ------------
        with tc.tile_pool(name="ln", bufs=3) as LN, \
             tc.tile_pool(name="ln_small", bufs=4) as LS, \
             tc.tile_pool(name="t_psum", bufs=2, space="PSUM") as TP, \
             tc.tile_pool(name="tb_psum", bufs=2, space="PSUM") as TPB, \
             tc.tile_pool(name="o_psum", bufs=2, space="PSUM") as OP, \
             tc.tile_pool(name="osb", bufs=3) as OS:
            for tb in range(NT_Q // 128):
                xt = LN.tile([128, KVE], f32, tag="ln_x")
                for c in range(4):
                    ps_t = TP.tile([128, 128], f32, tag="ps_t")
                    nc.tensor.transpose(
                        ps_t, xT[c][:, tb * 128:(tb + 1) * 128], ident)
                    if c % 2:
                        nc.vector.tensor_copy(
                            xt[:, c * 128:(c + 1) * 128], ps_t)
                    else:
                        nc.scalar.activation(
                            out=xt[:, c * 128:(c + 1) * 128], in_=ps_t,
                            func=ACTF.Identity)
                stats = LS.tile([128, 6], f32, tag="ln_stats")
                nc.vector.bn_stats(out=stats, in_=xt)
                mv = LS.tile([128, 2], f32, tag="ln_mv")
                nc.vector.bn_aggr(out=mv, in_=stats)
                sd = LS.tile([128, 1], f32, tag="ln_sd")
                nc.scalar.activation(
                    out=sd, in_=mv[:, 1:2], func=ACTF.Sqrt, bias=eps_col)
                rstd = LS.tile([128, 1], f32, tag="ln_rstd")
                nc.vector.reciprocal(rstd, sd)
                xn = LN.tile([128, KVE], f32, tag="ln_xn")
                # xn = (xt - mu)*rstd  as ACT: xt*rstd + (-mu*rstd)
                negmr = LS.tile([128, 1], f32, tag="ln_negmr")
                nc.vector.tensor_scalar(
                    negmr, mv[:, 0:1], rstd, -1.0, op0=ALU.mult, op1=ALU.mult)
                nc.scalar.activation(
                    out=xn, in_=xt, func=ACTF.Identity,
                    bias=negmr, scale=rstd)
                if has_gamma:
                    nc.vector.tensor_mul(xn, xn, gamma_bc)
                if has_beta:
                    nc.vector.tensor_add(xn, xn, beta_bc)
                # out act quant
                mx = LS.tile([128, 1], f32, tag="ln_mx")
                nc.vector.tensor_reduce(
                    mx, xn, axis=AX.X, op=ALU.max, apply_absolute_value=True)
                clip = LS.tile([128, 1], f32, tag="ln_clip")
                nc.vector.tensor_scalar(clip, mx, 1e-5, None, op0=ALU.max)
                nc.vector.tensor_scalar(
                    co_all[:, tb:tb + 1], clip, scl_bc[:, 3:4], None,
                    op0=ALU.mult)
                sx = LS.tile([128, 1], f32, tag="ln_sx")
                nc.vector.reciprocal(sx, clip)
                nc.vector.tensor_scalar(sx, sx, 127.0, None, op0=ALU.mult)
                t1 = LN.tile([128, KVE], f32, tag="ln_t1")
                nc.scalar.activation(
                    out=t1, in_=xn, func=ACTF.Identity,
                    bias=magic_col, scale=sx)
                # transpose rounded-int t1 (fp32-exact) via PE; the -MAGIC
                # subtract folds into the PSUM->SBUF copy
                xqoT = []
                for c in range(4):
                    ps_tb = TPB.tile([128, 128], f32, tag="ps_tb")
                    nc.tensor.transpose(
                        ps_tb, t1[:, c * 128:(c + 1) * 128], ident)
                    xq_c = OS.tile([128, 128], bf16, tag=f"xqoT{c}")
                    if c % 2:
                        nc.vector.tensor_scalar(
                            xq_c, ps_tb, -MAGIC, None, op0=ALU.add)
                    else:
                        nc.scalar.activation(
                            out=xq_c, in_=ps_tb, func=ACTF.Identity,
                            bias=negmagic_col)
                    xqoT.append(xq_c)
                for eb in range(4):
                    ps_o = OP.tile([128, 512], f32, tag="ps_o")
                    for c in range(4):
                        nc.tensor.matmul(
                            ps_o,
                            lhsT=xqoT[c],
                            rhs=woq[:, c, eb * 512:(eb + 1) * 512],
                            start=(c == 0), stop=(c == 3),
                        )
                    ot = OS.tile([128, 512], f32, tag="o_t")
                    if eb % 2:
                        nc.vector.tensor_scalar(
                            ot, ps_o, co_all[:, tb:tb + 1], None, op0=ALU.mult)
                    else:
                        nc.scalar.activation(
                            out=ot, in_=ps_o, func=ACTF.Identity,
                            scale=co_all[:, tb:tb + 1])
                    if has_bo:
                        nc.gpsimd.tensor_add(
                            ot, ot, bo_bc[:, eb * 512:(eb + 1) * 512])
                    (nc.sync if eb % 2 else nc.scalar).dma_start(
                        out=out_d[tb * 128:(tb + 1) * 128,
                                  eb * 512:(eb + 1) * 512],
                        in_=ot,
                    )

        if debug:
            with tc.tile_pool(name="dbg", bufs=2) as DB:
                for (src, dst) in ((kT[0], dbg_kT), (qT[0], dbg_qT),
                                   (vS[0], dbg_vS), (xT[0], dbg_xT)):
                    t = DB.tile([128, src.shape[-1]], f32, tag="dbg")
                    nc.vector.tensor_copy(t, src)
                    nc.sync.dma_start(out=dst, in_=t)

    nc.compile()
    return nc


def _get_nc(flags):
    key = ("nc", flags)
    if key not in _CACHE:
        _CACHE[key] = _build(flags)
    return _CACHE[key]


def _build_null():
    """Same external I/O signature, trivial body: calibrates the per-call
    dispatch floor of the current session."""
    import concourse.tile as tile
    import concourse.mybir as mybir
    from concourse import bacc

    f32 = mybir.dt.float32
    bf16 = mybir.dt.bfloat16
    f16 = mybir.dt.float16

    nc = bacc.Bacc(None, target_bir_lowering=False)
    nc.dram_tensor("q_in", [NT_Q, E], f16, kind="ExternalInput")
    nc.dram_tensor("k_in", [S, E], f16, kind="ExternalInput")
    nc.dram_tensor("v_in", [S, E], f16, kind="ExternalInput")
    nc.dram_tensor("wqs", [E, KVE], bf16, kind="ExternalInput")
    nc.dram_tensor("wkq", [E, KVE], bf16, kind="ExternalInput")
    nc.dram_tensor("wvq", [E, KVE], bf16, kind="ExternalInput")
    nc.dram_tensor("woq", [KVE, E], bf16, kind="ExternalInput")
    nc.dram_tensor("scl", [8], f32, kind="ExternalInput")
    nc.dram_tensor("bqs", [128, KH], f32, kind="ExternalInput")
    nc.dram_tensor("thr", [2, 512], f32, kind="ExternalInput")
    nc.dram_tensor("bv", [KVE], f32, kind="ExternalInput")
    nc.dram_tensor("bo", [E], f32, kind="ExternalInput")
    nc.dram_tensor("gamma", [KVE], f32, kind="ExternalInput")
    nc.dram_tensor("beta", [KVE], f32, kind="ExternalInput")
    out_d = nc.dram_tensor("out", [NT_Q, E], f32, kind="ExternalOutput").ap()
    with tile.TileContext(nc) as tc:
        with tc.tile_pool(name="p", bufs=1) as P:
            t = P.tile([128, E], f32, tag="t")
            nc.vector.memset(t, 0.0)
            nc.sync.dma_start(out=out_d[0:128, :], in_=t)
    nc.compile()
    return nc


def _get_null_nc():
    if "null" not in _CACHE:
        _CACHE["null"] = _build_null()
    return _CACHE["null"]


def _ternary(w):
    s = max(float(np.abs(w).mean()), 1e-5)
    return np.clip(np.round(w * (1.0 / s)), -1.0, 1.0), s


def _variant_key(inputs):
    return (
        bool(np.any(np.asarray(inputs["bv"]) != 0)),
        bool(np.any(np.asarray(inputs["bo"]) != 0)),
        bool(np.any(np.asarray(inputs["gamma"]) != 1)),
        bool(np.any(np.asarray(inputs["beta"]) != 0)),
    )


def _make_in_maps(inputs):
    query = np.asarray(inputs["query"], np.float32)
    key = np.asarray(inputs["key"], np.float32)
    value = np.asarray(inputs["value"], np.float32)
    Wq = np.asarray(inputs["Wq"], np.float32)
    Wk = np.asarray(inputs["Wk"], np.float32)
    Wv = np.asarray(inputs["Wv"], np.float32)
    Wo = np.asarray(inputs["Wo"], np.float32)
    bq = np.asarray(inputs["bq"], np.float32)

    # host weight quantization (static model prep)
    wq_t, s_q = _ternary(Wq)            # [E, E]
    wk_t, s_k = _ternary(Wk)            # [KVE, E]
    wv_t, s_v = _ternary(Wv)
    wo_t, s_o = _ternary(Wo)            # [E, KVE]
    # group-summed Q weights, transposed: [E_in, KH*HD]
    wqs = np.ascontiguousarray(
        wq_t.reshape(KH, QH // KH, HD, E).sum(axis=1)
        .reshape(KVE, E).T.astype(BF16))
    wkq = np.ascontiguousarray(wk_t.T.astype(BF16))     # [E, KVE]
    wvq = np.ascontiguousarray(wv_t.T.astype(BF16))
    woq = np.ascontiguousarray(wo_t.T.astype(BF16))     # [KVE, E]
    # bqs[d, h] = sum_g bq[(4h+g)*128+d] / 128
    bqs = np.ascontiguousarray(
        bq.reshape(KH, QH // KH, HD).sum(axis=1).T / 128.0
    ).astype(np.float32)
    scl = np.array(
        [s_q / (127.0 * 128.0), s_k / 127.0, s_v / 127.0, s_o / 127.0,
         0, 0, 0, 0], np.float32)

    qbf = query.astype(np.float16)
    kbf = key.astype(np.float16)
    vbf = value.astype(np.float16)

    in_maps = []
    for c in range(NCORES):
        b, half = c // 2, c % 2
        blocks = BLKS[half]
        q_rows = np.concatenate(
            [qbf[b, blk * 512:(blk + 1) * 512, :] for blk in blocks], axis=0)
        thr = np.stack(
            [blk * 512 + np.arange(512, dtype=np.float32) for blk in blocks])
        in_maps.append({
            "q_in": np.ascontiguousarray(q_rows),
            "k_in": np.ascontiguousarray(kbf[b]),
            "v_in": np.ascontiguousarray(vbf[b]),
            "wqs": wqs, "wkq": wkq, "wvq": wvq, "woq": woq,
            "scl": scl, "bqs": bqs,
            "thr": np.ascontiguousarray(thr),
            "bv": np.asarray(inputs["bv"], np.float32),
            "bo": np.asarray(inputs["bo"], np.float32),
            "gamma": np.asarray(inputs["gamma"], np.float32),
            "beta": np.asarray(inputs["beta"], np.float32),
        })
    return in_maps


def kernel(query, key, value, Wq, bq, Wk, bk, Wv, bv, Wo, bo, gamma, beta):
    from concourse.bass_utils import run_bass_kernel_spmd

    inputs = dict(query=query, key=key, value=value, Wq=Wq, bq=bq, Wk=Wk,
                  bk=bk, Wv=Wv, bv=bv, Wo=Wo, bo=bo, gamma=gamma, beta=beta)
    flags = _variant_key(inputs)
    nc = _get_nc(flags)
    in_maps = _make_in_maps(inputs)

    res = run_bass_kernel_spmd(nc, in_maps, core_ids=list(range(NCORES)))
    _CACHE["last_result"] = res

    out = np.zeros((B, S, E), np.float32)
    for c in range(NCORES):
        b, half = c // 2, c % 2
        blocks = BLKS[half]
        o = res.results[c]["out"]
        for i, blk in enumerate(blocks):
            out[b, blk * 512:(blk + 1) * 512, :] = o[i * 512:(i + 1) * 512, :]
    return out
